# revision 6
# baseline (speedup 1.0000x reference)
"""Trainium2 Bass kernel for nn_Coref_Injection (ragged coref head).

Sharding: pure data-parallel — 8 cores = 4 batches x 2 pair-halves.
Core c handles batch b=c//2, pair range [h*1024, (h+1)*1024) with h=c%2
(mention range [h*64, (h+1)*64)); MLP weights replicated.

Per-core dataflow (transposed layout, fp16 matmuls, fp32 accumulation):
  head/tail [1024,2048] fp32  --DMA-->  SBUF --DVE cast--> fp16
      --PE transpose (128x128 blocks)--> headT/tailT [t,p] chunks
  hrT[d,p] = x[t,d].T-contract @ headT   (PSUM fp32, k=t in 16 chunks)
  featT = {hrT, trT, hrT*trT};  hT = relu(w1.T@featT + b1);  logitsT = w2.T@hT + b2
  scores free-major -> per-segment max -> one-hot*max (g) -> gt = trT*g
      -> segment-sum = crmT -> PE transpose -> crm [64,256]
  coref_rep[t,d] = cmp[m,t].T @ crm  (+ x on even cores)
  loss: logits transposed to partition-major [128,16], masked KL partial sum.
Host combines: out[b] = partial[2b]+partial[2b+1] (+x via addx flag on core 2b),
loss = sum(pw)/2/sum(mask).
"""

import numpy as np

import concourse.bacc as bacc
import concourse.bass as bass
import concourse.mybir as mybir
import concourse.tile as tile
from concourse.bass_utils import run_bass_kernel_spmd

B, T, D, M, L = 4, 2048, 256, 128, 16
P = M * L
NCORES = 8
PH = P // 2      # pairs per core
MH = M // 2      # mentions per core
NK = T // 128    # 16 t-chunks
F32 = mybir.dt.float32
F16 = mybir.dt.float16

AF = mybir.ActivationFunctionType
ALU = mybir.AluOpType
AX = mybir.AxisListType


def build_kernel(nc):
    head_d = nc.dram_tensor("head", [PH, T], F32, kind="ExternalInput")
    tail_d = nc.dram_tensor("tail", [PH, T], F32, kind="ExternalInput")
    x_d = nc.dram_tensor("x", [T, D], F32, kind="ExternalInput")
    cmp_d = nc.dram_tensor("cmp", [MH, T], F32, kind="ExternalInput")
    lab_d = nc.dram_tensor("lab", [PH, 2], F32, kind="ExternalInput")
    mask_d = nc.dram_tensor("mask", [PH], F32, kind="ExternalInput")
    w1_d = nc.dram_tensor("w1", [3 * D, D], F32, kind="ExternalInput")
    b1_d = nc.dram_tensor("b1", [D], F32, kind="ExternalInput")
    w2_d = nc.dram_tensor("w2", [D, 2], F32, kind="ExternalInput")
    b2_d = nc.dram_tensor("b2", [2], F32, kind="ExternalInput")
    id16_d = nc.dram_tensor("id16", [128, 128], F16, kind="ExternalInput")
    id32_d = nc.dram_tensor("id32", [128, 128], F32, kind="ExternalInput")
    ones16_d = nc.dram_tensor("ones16", [1, 128], F16, kind="ExternalInput")
    ones32_d = nc.dram_tensor("ones32", [128], F32, kind="ExternalInput")
    addx_d = nc.dram_tensor("addx", [128], F32, kind="ExternalInput")

    outp_d = nc.dram_tensor("outp", [T, D], F32, kind="ExternalOutput")
    pw_d = nc.dram_tensor("pw", [1, 1], F32, kind="ExternalOutput")

    with tile.TileContext(nc) as tc:
        _body(tc, nc, head_d, tail_d, x_d, cmp_d, lab_d, mask_d, w1_d, b1_d,
              w2_d, b2_d, id16_d, id32_d, ones16_d, ones32_d, addx_d,
              outp_d, pw_d)
    nc.compile()
    return nc


def _body(tc, nc, head_d, tail_d, x_d, cmp_d, lab_d, mask_d, w1_d, b1_d,
          w2_d, b2_d, id16_d, id32_d, ones16_d, ones32_d, addx_d,
          outp_d, pw_d):
    act, dve, pe, sync = nc.scalar, nc.vector, nc.tensor, nc.sync

    pools = []

    def mkpool(**kw):
        p = tc.alloc_tile_pool(**kw)
        pools.append(p)
        return p

    ld = mkpool(name="ld", bufs=3)           # fp32 streaming loads
    ldf16 = mkpool(name="ldf16", bufs=3)     # fp16 casts of streams
    const = mkpool(name="const", bufs=1)     # persistent small tiles
    chunks = mkpool(name="chunks", bufs=18)  # transposed t-chunks
    outpool = mkpool(name="outpool", bufs=2)
    pacc = mkpool(name="pacc", bufs=4, space="PSUM")
    ptr = mkpool(name="ptr", bufs=2, space="PSUM")
    pmisc = mkpool(name="pmisc", bufs=2, space="PSUM")

    # ---- constants & small loads ----
    id16 = const.tile([128, 128], F16, tag="id16")
    sync.dma_start(id16[:], id16_d.ap())
    id32 = const.tile([128, 128], F32, tag="id32")
    sync.dma_start(id32[:], id32_d.ap())
    ones16 = const.tile([1, 128], F16, tag="ones16")
    sync.dma_start(ones16[:], ones16_d.ap())
    ones32 = const.tile([128, 1], F32, tag="ones32")
    sync.dma_start(ones32[:], ones32_d.ap().rearrange("(q o) -> q o", o=1))
    addx = const.tile([128, 1], F32, tag="addx")
    sync.dma_start(addx[:], addx_d.ap().rearrange("(q o) -> q o", o=1))

    # x: [T, D] -> 2 supertiles [128, 8*256]; chunk k at cols (k%8)*256
    x32 = []
    x16 = []
    for i in range(2):
        x32_t = const.tile([128, 8 * D], F32, tag=f"x32_{i}")
        sync.dma_start(
            x32_t[:].rearrange("q (c d) -> q c d", d=D),
            x_d.ap().rearrange("(c q) d -> q c d", q=128)[:, 8 * i:8 * (i + 1), :])
        x16_t = const.tile([128, 8 * D], F16, tag=f"x16_{i}")
        dve.tensor_copy(x16_t[:], x32_t[:])
        x32.append(x32_t)
        x16.append(x16_t)

    # w1: [768, 256] -> [128, 6*256] fp16
    w1_32 = ld.tile([128, 6 * D], F32, tag="ld")
    sync.dma_start(
        w1_32[:].rearrange("q (c d) -> q c d", d=D),
        w1_d.ap().rearrange("(c q) d -> q c d", q=128))
    w1_16 = const.tile([128, 6 * D], F16, tag="w1_16")
    dve.tensor_copy(w1_16[:], w1_32[:])

    # w2: [256, 2] -> [128, 4] fp16 (chunk kc at cols kc*2)
    w2_32 = const.tile([128, 4], F32, tag="w2_32")
    sync.dma_start(
        w2_32[:].rearrange("q (c t) -> q c t", t=2),
        w2_d.ap().rearrange("(c q) t -> q c t", q=128))
    w2_16 = const.tile([128, 4], F16, tag="w2_16")
    dve.tensor_copy(w2_16[:], w2_32[:])

    b1_t = const.tile([128, 2], F32, tag="b1_t")
    sync.dma_start(b1_t[:], b1_d.ap().rearrange("(c q) -> q c", q=128))
    b2_t = const.tile([2, 1], F32, tag="b2_t")
    sync.dma_start(b2_t[:], b2_d.ap().rearrange("(q o) -> q o", o=1))

    # cmp: [64, 2048] fp32 -> fp16
    cmp32 = ld.tile([MH, T], F32, tag="ld")
    sync.dma_start(cmp32[:], cmp_d.ap())
    cmp16 = const.tile([MH, T], F16, tag="cmp16")
    dve.tensor_copy(cmp16[:], cmp32[:])

    # lab/mask partition-major: (q, j) <-> pair p = j*128+q
    lab_t = const.tile([128, 16], F32, tag="lab_t")
    sync.dma_start(
        lab_t[:].rearrange("q (j c) -> q j c", c=2),
        lab_d.ap().rearrange("(j q) c -> q j c", q=128))
    mask_t = const.tile([128, 8], F32, tag="mask_t")
    sync.dma_start(mask_t[:], mask_d.ap().rearrange("(j q) -> q j", q=128))

    # ---- persistent fp16 activations (reps in [d, p] layout) ----
    hrT = [const.tile([128, PH], F16, tag=f"hrT{m}", name=f"hrT{m}") for m in range(2)]
    trT = [const.tile([128, PH], F16, tag=f"trT{m}", name=f"trT{m}") for m in range(2)]
    prodT = [const.tile([128, PH], F16, tag=f"prodT{m}", name=f"prodT{m}") for m in range(2)]
    hrelu = [const.tile([128, PH], F16, tag=f"hrelu{m}", name=f"hrelu{m}") for m in range(2)]
    logitsT = const.tile([2, PH], F32, tag="logitsT")

    big_view = {
        "h": head_d.ap().rearrange("(s c q) t -> s q c t", q=128, c=2),
        "t": tail_d.ap().rearrange("(s c q) t -> s q c t", q=128, c=2),
    }
    rep_sb = {"h": hrT, "t": trT}

    # ---- main reps pipeline ----
    for pb in range(2):            # p-blocks of 512
        for tsr in ("h", "t"):
            f16s = []
            for si in range(2):    # two 2MB supertiles cover this p-block
                s = 2 * pb + si
                st32 = ld.tile([128, 2 * T], F32, tag="ld")
                sync.dma_start(
                    st32[:].rearrange("q (c t) -> q c t", t=T),
                    big_view[tsr][s])
                st16 = ldf16.tile([128, 2 * T], F16, tag="ldf16")
                dve.tensor_copy(st16[:], st32[:])
                f16s.append(st16)
            ch = []
            for k in range(NK):
                ptile = ptr.tile([128, 512], F16, tag="ptr")
                for sub in range(4):     # p-subtile within block
                    src = f16s[sub // 2][:, (sub % 2) * T + k * 128:
                                         (sub % 2) * T + (k + 1) * 128]
                    pe.transpose(ptile[:, sub * 128:(sub + 1) * 128], src, id16[:])
                cht = chunks.tile([128, 512], F16, tag=f"ch_{tsr}")
                act.copy(cht[:], ptile[:])
                ch.append(cht)
            for m in range(2):
                rpsum = pacc.tile([128, 512], F32, tag="acc", name=f"racc{tsr}{m}{pb}")
                for k in range(NK):
                    xsl = x16[k // 8][:, (k % 8) * D + m * 128:
                                      (k % 8) * D + m * 128 + 128]
                    pe.matmul(rpsum[:], xsl, ch[k][:],
                              start=(k == 0), stop=(k == NK - 1))
                act.copy(rep_sb[tsr][m][:, pb * 512:(pb + 1) * 512], rpsum[:])

        # featT product chunk + MLP for this p-block
        for m in range(2):
            dve.tensor_mul(prodT[m][:, pb * 512:(pb + 1) * 512],
                           hrT[m][:, pb * 512:(pb + 1) * 512],
                           trT[m][:, pb * 512:(pb + 1) * 512])
        feat = [hrT[0], hrT[1], trT[0], trT[1], prodT[0], prodT[1]]
        for m in range(2):
            hpsum = pacc.tile([128, 512], F32, tag="acc", name=f"hacc{m}{pb}")
            for kc in range(6):
                w1sl = w1_16[:, kc * D + m * 128:kc * D + m * 128 + 128]
                pe.matmul(hpsum[:], w1sl, feat[kc][:, pb * 512:(pb + 1) * 512],
                          start=(kc == 0), stop=(kc == 5))
            act.activation(hrelu[m][:, pb * 512:(pb + 1) * 512], hpsum[:],
                           AF.Relu, bias=b1_t[:, m:m + 1])
        lgpsum = pmisc.tile([2, 512], F32, tag="pmisc")
        for kc in range(2):
            pe.matmul(lgpsum[:], w2_16[:, kc * 2:(kc + 1) * 2],
                      hrelu[kc][:, pb * 512:(pb + 1) * 512],
                      start=(kc == 0), stop=(kc == 1))
        act.activation(logitsT[:, pb * 512:(pb + 1) * 512], lgpsum[:],
                       AF.Identity, bias=b2_t[:])

    # ---- select: per-segment max -> g = max * onehot ----
    # class columns are swapped host-side, so class-1 scores sit on partition 0
    scores = logitsT[0:1, :]                       # [1, 1024] fp32
    segmax = const.tile([1, MH], F32, tag="segmax")
    dve.tensor_reduce(segmax[:], scores.rearrange("o (s l) -> o s l", l=L),
                      axis=AX.X, op=ALU.max)
    segmax_b = const.tile([1, PH], F32, tag="segmax_b")
    dve.tensor_copy(segmax_b[:].rearrange("o (s l) -> o s l", l=L),
                    segmax[:].unsqueeze(-1).broadcast_to((1, MH, L)))
    g32 = const.tile([1, PH], F32, tag="g32")
    dve.tensor_tensor(g32[:], scores, segmax_b[:], op=ALU.is_equal)
    dve.tensor_mul(g32[:], g32[:], segmax_b[:])
    g16 = const.tile([1, PH], F16, tag="g16")
    dve.tensor_copy(g16[:], g32[:])

    # broadcast g over 128 partitions via ones[1,128].T @ g
    g_b = const.tile([128, PH], F16, tag="g_b")
    for pb in range(2):
        bpsum = pmisc.tile([128, 512], F32, tag="pmisc")
        pe.matmul(bpsum[:], ones16[:], g16[:, pb * 512:(pb + 1) * 512],
                  start=True, stop=True)
        act.copy(g_b[:, pb * 512:(pb + 1) * 512], bpsum[:])

    # crmT[d, m] = segsum(trT * g_b); then transpose to crm [64, 256]
    crm = const.tile([MH, 2 * 128], F16, tag="crm")
    for m in range(2):
        gt = const.tile([128, PH], F16, tag=f"gt{m}")
        dve.tensor_mul(gt[:], trT[m][:], g_b[:])
        crmT32 = const.tile([128, MH], F32, tag=f"crmT32_{m}")
        dve.tensor_reduce(crmT32[:], gt[:].rearrange("q (s l) -> q s l", l=L),
                          axis=AX.X, op=ALU.add)
        crmT16 = const.tile([128, MH], F16, tag=f"crmT16_{m}")
        dve.tensor_copy(crmT16[:], crmT32[:])
        cpsum = pmisc.tile([MH, 128], F16, tag="pmisc")
        pe.transpose(cpsum[:], crmT16[:], id16[:])
        act.copy(crm[:, m * 128:(m + 1) * 128], cpsum[:])

    # ---- scatter: coref[t,d] = cmp.T @ crm ; out = addx*x + coref ----
    for k4 in range(4):
        ostage = outpool.tile([128, 4 * D], F32, tag="ostage")
        for kk in range(4):
            k = 4 * k4 + kk
            spsum = pmisc.tile([128, D], F32, tag="pmisc")
            pe.matmul(spsum[:], cmp16[:, k * 128:(k + 1) * 128], crm[:],
                      start=True, stop=True)
            xsl = x32[k // 8][:, (k % 8) * D:(k % 8) * D + D]
            dve.scalar_tensor_tensor(
                ostage[:, kk * D:(kk + 1) * D], xsl, addx[:, 0:1], spsum[:],
                op0=ALU.mult, op1=ALU.add)
        sync.dma_start(
            outp_d.ap().rearrange("(c q) d -> q c d", q=128)[:, 4 * k4:4 * (k4 + 1), :],
            ostage[:].rearrange("q (c d) -> q c d", d=D))

    # ---- loss: transpose logits to partition-major [128, 16] ----
    lg_pm = const.tile([128, 16], F32, tag="lg_pm")
    for j in range(8):
        tpsum = pmisc.tile([128, 2], F32, tag="pmisc")
        pe.transpose(tpsum[:], logitsT[0:2, j * 128:(j + 1) * 128], id32[0:2, 0:2])
        act.copy(lg_pm[:, 2 * j:2 * j + 2], tpsum[:])

    pair3 = lambda t: t[:].rearrange("q (j c) -> q j c", c=2)
    mx = const.tile([128, 8], F32, tag="mx")
    dve.tensor_reduce(mx[:], pair3(lg_pm), axis=AX.X, op=ALU.max)
    mx_b = mx[:].unsqueeze(-1).broadcast_to((128, 8, 2))
    sh = const.tile([128, 16], F32, tag="sh")
    dve.tensor_tensor(pair3(sh), pair3(lg_pm), mx_b, op=ALU.subtract)
    ex = const.tile([128, 16], F32, tag="ex")
    act.activation(ex[:], sh[:], AF.Exp)
    sm = const.tile([128, 8], F32, tag="sm")
    dve.tensor_reduce(sm[:], pair3(ex), axis=AX.X, op=ALU.add)
    lse = const.tile([128, 8], F32, tag="lse")
    act.activation(lse[:], sm[:], AF.Ln)
    dve.tensor_add(lse[:], lse[:], mx[:])
    lse_b = lse[:].unsqueeze(-1).broadcast_to((128, 8, 2))
    logq = const.tile([128, 16], F32, tag="logq")
    dve.tensor_tensor(pair3(logq), pair3(lg_pm), lse_b, op=ALU.subtract)
    labc = const.tile([128, 16], F32, tag="labc")
    dve.tensor_scalar_max(labc[:], lab_t[:], 1e-38)
    ll = const.tile([128, 16], F32, tag="ll")
    act.activation(ll[:], labc[:], AF.Ln)
    dve.tensor_sub(ll[:], ll[:], logq[:])
    dve.tensor_mul(ll[:], ll[:], lab_t[:])
    mask_b = mask_t[:].unsqueeze(-1).broadcast_to((128, 8, 2))
    dve.tensor_tensor(pair3(ll), pair3(ll), mask_b, op=ALU.mult)
    psum_red = const.tile([128, 1], F32, tag="psum_red")
    dve.tensor_reduce(psum_red[:], ll[:], axis=AX.X, op=ALU.add)
    lpsum = pmisc.tile([1, 1], F32, tag="pmisc")
    pe.matmul(lpsum[:], psum_red[:], ones32[:], start=True, stop=True)
    pw_sb = const.tile([1, 1], F32, tag="pw_sb")
    act.copy(pw_sb[:], lpsum[:])
    sync.dma_start(pw_d.ap(), pw_sb[:])

    for p in reversed(pools):
        p.release()


_CACHED_NC = None


def _get_nc():
    global _CACHED_NC
    if _CACHED_NC is None:
        nc = bacc.Bacc("TRN2", target_bir_lowering=False, debug=False)
        _CACHED_NC = build_kernel(nc)
    return _CACHED_NC


def make_in_maps(inputs):
    head = np.asarray(inputs["head"], np.float32)
    tail = np.asarray(inputs["tail"], np.float32)
    x = np.asarray(inputs["x"], np.float32)
    cmp_ = np.asarray(inputs["coref_mention_position"], np.float32)
    lab = np.asarray(inputs["coref_label"], np.float32)
    mask = np.asarray(inputs["coref_label_mask"]).astype(np.float32)
    w1 = np.asarray(inputs["w1"], np.float32)
    b1 = np.asarray(inputs["b1"], np.float32)
    w2 = np.asarray(inputs["w2"], np.float32)
    b2 = np.asarray(inputs["b2"], np.float32)
    id16 = np.eye(128, dtype=np.float16)
    id32 = np.eye(128, dtype=np.float32)
    ones16 = np.ones((1, 128), np.float16)
    ones32 = np.ones((128,), np.float32)
    in_maps = []
    for c in range(NCORES):
        b, h = c // 2, c % 2
        sl = slice(h * PH, (h + 1) * PH)
        msl = slice(h * MH, (h + 1) * MH)
        in_maps.append({
            "head": np.ascontiguousarray(head[b, sl]),
            "tail": np.ascontiguousarray(tail[b, sl]),
            "x": np.ascontiguousarray(x[b]),
            "cmp": np.ascontiguousarray(cmp_[b, msl]),
            "lab": np.ascontiguousarray(lab[b, sl, ::-1]),
            "mask": np.ascontiguousarray(mask[b, sl]),
            "w1": w1, "b1": b1,
            "w2": np.ascontiguousarray(w2[:, ::-1]),
            "b2": np.ascontiguousarray(b2[::-1]),
            "id16": id16, "id32": id32, "ones16": ones16, "ones32": ones32,
            "addx": np.full((128,), 1.0 - h, np.float32),
        })
    return in_maps


def run(inputs, trace=False, **kw):
    nc = _get_nc()
    in_maps = make_in_maps(inputs)
    res = run_bass_kernel_spmd(nc, in_maps, list(range(NCORES)), trace=trace, **kw)
    return res


def assemble(inputs, results):
    mask = np.asarray(inputs["coref_label_mask"])
    out = np.empty((B, T, D), np.float32)
    pw_total = 0.0
    for c in range(NCORES):
        b, h = c // 2, c % 2
        part = results[c]["outp"]
        if h == 0:
            out[b] = part
        else:
            out[b] += part
        pw_total += float(results[c]["pw"][0, 0])
    n = float(mask.sum())
    loss = np.float32(pw_total / (n * 2.0))
    return out, loss


def kernel(**inputs):
    res = run(inputs)
    return assemble(inputs, res.results)
